# revision 8
# baseline (speedup 1.0000x reference)
"""Trainium2 Bass kernel for nn_ContextualViewModel_48833778155979.

Computation (see reference):
    station_feats = x[sx, sy]            # (K, F) gather -- on host (hint: replicate)
    y = station_feats @ W                # (K, F) tiny matmul (0.05% of FLOPs) -- host
    res[h, w, :] = sum_k d[h, w, k] * y[k, :]   # big (H*W, K) @ (K, F) matmul -- device

Sharding: H axis split across 8 cores (48 rows each -> 18432 grid cells/core).
Per core the big matmul is (18432, 256) @ (256, 256).

The kernel is HBM-DMA bound: 16 DMA engines/core at ~26.5 B/ns each with
8 KiB lines. All device I/O is fp16 (halves bytes vs fp32; adds ~5e-4
rel err, far under the 1e-2 gate): d is pre-transposed on the host into
a k-major slab-packed layout with 8 KiB/partition DMA lines, the output
is stored fp16 with 8 KiB lines and upcast on host.

Per-core device loop (9 slabs of 2048 grid cells):
  - slab DMA in:  [128 kp, 2 kc, 2048 r] fp16 (8 KiB line/partition)
  - 8 PSUM groups of 2 subtiles: 4 fp16 matmuls (d-chunk stationary,
    y moving, N=256) accumulating over the two 128-wide k chunks
  - one PSUM->SBUF fp16 copy per group, alternating vector/scalar
    (amortizes the ~290ns fixed PSUM access cost)
  - slab DMA out (trigger on the otherwise-idle gpsimd engine):
    [128 p, 16 n, 256 f] fp16, 8 KiB line/partition, row order chosen
    so HBM rows land in natural order.
"""

import sys

sys.path.insert(0, "/opt/trn_rl_repo")

from contextlib import ExitStack

import numpy as np

import concourse.bacc as bacc
import concourse.mybir as mybir
import concourse.tile as tile
from concourse.bass_utils import run_bass_kernel_spmd

H, WG, F = 384, 384, 256
K = 256
NCORES = 8
HS = H // NCORES          # 48 grid rows per core
ROWS = HS * WG            # 18432 cells per core
RS = 2048                 # rows per DMA slab
NSLAB = ROWS // RS        # 9
NSUB = RS // 128          # 16 subtiles of 128 rows per slab

F16 = mybir.dt.float16
F32 = mybir.dt.float32

_cache: dict = {}
last_results = None  # BassKernelResults of the most recent kernel() call


def _build_program(reps: int = 1):
    key = ("nc", reps)
    if key in _cache:
        return _cache[key]

    nc = bacc.Bacc(
        "TRN2", target_bir_lowering=False, debug=False, num_devices=NCORES
    )

    # d, pre-transposed/packed on host: row s*128+kp holds, for slab s and
    # k-part kp, the 2*RS fp16 values [kc, n, p] = d[s*RS + p*16 + n, kc*128+kp]
    d_ext = nc.dram_tensor("d_t", [NSLAB * 128, 2 * RS], F16, kind="ExternalInput").ap()
    # y packed host-side as [kp, kc, f] so each partition is one 1 KiB line
    y_ext = nc.dram_tensor("y_mat", [128, 2 * F], F16, kind="ExternalInput").ap()
    out_ext = nc.dram_tensor("out_shard", [ROWS, F], F16, kind="ExternalOutput").ap()

    with tile.TileContext(nc) as tc, ExitStack() as ctx:
        const = ctx.enter_context(tc.tile_pool(name="const", bufs=1))
        dpool = ctx.enter_context(tc.tile_pool(name="din", bufs=NSLAB))
        opool = ctx.enter_context(tc.tile_pool(name="dout", bufs=5))
        mpsum = ctx.enter_context(tc.tile_pool(name="mpsum", bufs=3, space="PSUM"))

        din_tiles = {}

        def issue_din(s):
            t = dpool.tile([128, 2, RS], F16, tag="din")
            nc.sync.dma_start(
                t[:, :, :],
                d_ext[s * 128 : (s + 1) * 128, :].rearrange(
                    "p (kc r) -> p kc r", kc=2
                ),
            )
            din_tiles[s] = t

        # Front-load the entire input stream: first d slab starts
        # transferring before anything else, y (tiny) lands while it
        # streams, and the remaining slabs queue up behind so the DMA
        # engines never idle waiting on buffer recycling.
        issue_din(0)
        y_sb = const.tile([128, 2, F], F16)
        nc.sync.dma_start(
            y_sb[:, :, :], y_ext.rearrange("p (kc f) -> p kc f", kc=2)
        )
        for s in range(1, NSLAB):
            issue_din(s)

        def emit_slab(s):
            din = din_tiles.pop(s)
            dout = opool.tile([128, NSUB, F], F16, tag="dout")
            # 4 PSUM groups of 4 subtiles (2 banks each); one copy per group
            # alternating vector/scalar; output DMA per half-slab so the
            # store stream starts draining mid-slab.
            for g in range(NSUB // 4):
                po = mpsum.tile([128, 4, F], F32, tag="po")
                for j in range(4):
                    n = 4 * g + j
                    nc.tensor.matmul(
                        po[:, j, :],
                        din[:, 0, n * 128 : (n + 1) * 128],
                        y_sb[:, 0, :],
                        start=True,
                        stop=False,
                    )
                    nc.tensor.matmul(
                        po[:, j, :],
                        din[:, 1, n * 128 : (n + 1) * 128],
                        y_sb[:, 1, :],
                        start=False,
                        stop=True,
                    )
                if g % 2 == 0:
                    nc.vector.tensor_copy(dout[:, 4 * g : 4 * g + 4, :], po[:, :, :])
                else:
                    nc.scalar.copy(dout[:, 4 * g : 4 * g + 4, :], po[:, :, :])
                if g % 2 == 1:
                    h = g // 2
                    nc.gpsimd.dma_start(
                        out_ext[s * RS : (s + 1) * RS, :].rearrange(
                            "(p hh n) f -> hh p n f", hh=2, n=NSUB // 2
                        )[h],
                        dout[:, h * (NSUB // 2) : (h + 1) * (NSUB // 2), :],
                    )

        def emit_pipeline():
            for s in range(NSLAB):
                emit_slab(s)

        if reps == 1:
            emit_pipeline()
        else:
            with tc.For_i(0, reps, 1):
                emit_pipeline()

    nc.compile()
    _cache[key] = nc
    return nc


def kernel(x, d, W, sx, sy):
    x = np.asarray(x, dtype=np.float32)
    d = np.asarray(d, dtype=np.float32)
    W = np.asarray(W, dtype=np.float32)
    sx = np.asarray(sx, dtype=np.int32)
    sy = np.asarray(sy, dtype=np.int32)

    # Host-side gather of the K station feature vectors + the tiny (K,F)@(F,F)
    # matmul (replicated to all cores per the sharding strategy), packed
    # [kp, kc, f] for single-line-per-partition DMA.
    y16 = (x[sx, sy] @ W).astype(np.float16)
    y16 = np.ascontiguousarray(
        y16.reshape(2, 128, F).transpose(1, 0, 2)
    ).reshape(128, 2 * F)

    # Pack d k-major per core: dt[c, s, kp, kc, n, p] = d-row (s*RS + p*16 + n)
    d16 = d.astype(np.float16)
    dv = d16.reshape(NCORES, NSLAB, 128, NSUB, 2, 128)  # [c, s, p, n, kc, kp]
    dt = np.ascontiguousarray(dv.transpose(0, 1, 5, 4, 3, 2))

    nc = _build_program()

    in_maps = []
    for c in range(NCORES):
        in_maps.append(
            {
                "d_t": dt[c].reshape(NSLAB * 128, 2 * RS),
                "y_mat": y16,
            }
        )

    res = run_bass_kernel_spmd(nc, in_maps, list(range(NCORES)))
    global last_results
    last_results = res
    out = np.concatenate(
        [r["out_shard"].reshape(HS, WG, F) for r in res.results], axis=0
    ).astype(np.float32)
    return out


if __name__ == "__main__":
    rng = np.random.default_rng(0)
    x = rng.standard_normal((H, WG, F), dtype=np.float32)
    d = rng.random((H, WG, K), dtype=np.float32)
    W = rng.standard_normal((K, F), dtype=np.float32) / np.sqrt(F)
    sx = rng.integers(0, H, size=(K,)).astype(np.int32)
    sy = rng.integers(0, WG, size=(K,)).astype(np.int32)
    out = kernel(x, d, W, sx, sy)
    y = x[sx, sy].astype(np.float64) @ W.astype(np.float64)
    exp = d.reshape(-1, K).astype(np.float64) @ y
    exp = exp.reshape(H, WG, F)
    err = np.linalg.norm(out - exp) / np.linalg.norm(exp)
    print("rel err:", err)


# revision 9
# speedup vs baseline: 1.1200x; 1.1200x over previous
"""Trainium2 Bass kernel for nn_ContextualViewModel_48833778155979.

Computation (see reference):
    station_feats = x[sx, sy]            # (K, F) gather -- on host (hint: replicate)
    y = station_feats @ W                # (K, F) tiny matmul (0.05% of FLOPs) -- host
    res[h, w, :] = sum_k d[h, w, k] * y[k, :]   # big (H*W, K) @ (K, F) matmul -- device

Sharding: H axis split across 8 cores (48 rows each -> 18432 grid cells/core).
Per core the big matmul is (18432, 256) @ (256, 256).

The kernel is HBM-DMA bound: 16 DMA engines/core at ~26.5 B/ns each
(~400 GB/s). All device I/O is fp16 (halves bytes vs fp32; adds ~5e-4
rel err, far under the 1e-2 gate): d is pre-transposed on the host into
a k-major layout with 8 KiB/partition DMA lines; the output is stored
fp16 f-major and transposed/upcast on the host.

Device structure per core (9 slabs of 2048 grid cells, fully prefetched
into SBUF so the DMA engines never wait on buffer recycling):
  - the tiny y is the matmul STATIONARY operand (4 distinct 128x128
    tiles) and d streams as the MOVING operand at N=512, which halves
    the matmul/ldweights instruction count vs d-stationary and keeps
    the tensor engine (the mid-run pacer) ahead of the DMA stream;
  - PSUM [128, 512] accumulates over the two 128-wide k chunks, then
    one copy per bank (alternating vector/scalar engines) casts to an
    fp16 [128 f, 2 fc, 2048 r] staging tile;
  - slab output DMA (trigger on the otherwise-idle gpsimd engine) with
    4 KiB/partition lines;
  - slab 0's input is split into four 512-row DMAs so the first matmul
    starts ~2.5us earlier and the PE ramps while the rest streams.
"""

import sys

sys.path.insert(0, "/opt/trn_rl_repo")

from contextlib import ExitStack

import numpy as np

import concourse.bacc as bacc
import concourse.mybir as mybir
import concourse.tile as tile
from concourse.bass_utils import run_bass_kernel_spmd

H, WG, F = 384, 384, 256
K = 256
NCORES = 8
HS = H // NCORES          # 48 grid rows per core
ROWS = HS * WG            # 18432 cells per core
RS = 2048                 # rows per DMA slab
NSLAB = ROWS // RS        # 9
GR = 512                  # rows per matmul group (= one PSUM bank of fp32)
NG = RS // GR             # 4 groups per slab

F16 = mybir.dt.float16
F32 = mybir.dt.float32

_cache: dict = {}
last_results = None  # BassKernelResults of the most recent kernel() call


def _build_program(reps: int = 1):
    key = ("nc", reps)
    if key in _cache:
        return _cache[key]

    nc = bacc.Bacc(
        "TRN2", target_bir_lowering=False, debug=False, num_devices=NCORES
    )

    # d, pre-transposed on host: row s*128+kp holds the 2*RS fp16 values
    # [kc, r] = d[s*RS + r, kc*128 + kp]
    d_ext = nc.dram_tensor("d_t", [NSLAB * 128, 2 * RS], F16, kind="ExternalInput").ap()
    # y packed host-side as [kp, kc, f] so each partition is one 1 KiB line
    y_ext = nc.dram_tensor("y_mat", [128, 2 * F], F16, kind="ExternalInput").ap()
    # output f-major: out[f, s*RS + r] (host transposes back)
    out_ext = nc.dram_tensor("out_shard", [F, ROWS], F16, kind="ExternalOutput").ap()

    with tile.TileContext(nc) as tc, ExitStack() as ctx:
        const = ctx.enter_context(tc.tile_pool(name="const", bufs=1))
        dpool = ctx.enter_context(tc.tile_pool(name="din", bufs=NSLAB))
        opool = ctx.enter_context(tc.tile_pool(name="dout", bufs=4))
        mpsum = ctx.enter_context(tc.tile_pool(name="mpsum", bufs=4, space="PSUM"))

        din_tiles = {}

        # Front-load the entire input stream: y (tiny, needed first) then
        # slab 0 in four 512-row pieces, then the remaining slabs.
        y_sb = const.tile([128, 2, F], F16)
        nc.sync.dma_start(
            y_sb[:, :, :], y_ext.rearrange("p (kc f) -> p kc f", kc=2)
        )
        t0 = dpool.tile([128, 2, RS], F16, tag="din")
        src0 = d_ext[0:128, :].rearrange("p (kc r) -> p kc r", kc=2)
        for q in range(NG):
            nc.sync.dma_start(
                t0[:, :, q * GR : (q + 1) * GR], src0[:, :, q * GR : (q + 1) * GR]
            )
        din_tiles[0] = t0
        for s in range(1, NSLAB):
            t = dpool.tile([128, 2, RS], F16, tag="din")
            nc.sync.dma_start(
                t[:, :, :],
                d_ext[s * 128 : (s + 1) * 128, :].rearrange(
                    "p (kc r) -> p kc r", kc=2
                ),
            )
            din_tiles[s] = t

        def emit_slab(s):
            din = din_tiles.pop(s)
            dout = opool.tile([128, 2, RS], F16, tag="dout")
            for g in range(NG):
                for fc in range(2):
                    po = mpsum.tile([128, GR], F32, tag="po")
                    for kc in range(2):
                        nc.tensor.matmul(
                            po[:, :],
                            y_sb[:, kc, fc * 128 : (fc + 1) * 128],
                            din[:, kc, g * GR : (g + 1) * GR],
                            start=(kc == 0),
                            stop=(kc == 1),
                        )
                    if (g + fc) % 2 == 0:
                        nc.vector.tensor_copy(
                            dout[:, fc, g * GR : (g + 1) * GR], po[:, :]
                        )
                    else:
                        nc.scalar.copy(
                            dout[:, fc, g * GR : (g + 1) * GR], po[:, :]
                        )
            nc.gpsimd.dma_start(
                out_ext[:, s * RS : (s + 1) * RS].rearrange(
                    "(fc fp) r -> fp fc r", fc=2
                ),
                dout[:, :, :],
            )

        def emit_pipeline():
            for s in range(NSLAB):
                emit_slab(s)

        if reps == 1:
            emit_pipeline()
        else:
            with tc.For_i(0, reps, 1):
                emit_pipeline()

    nc.compile()
    _cache[key] = nc
    return nc


def kernel(x, d, W, sx, sy):
    x = np.asarray(x, dtype=np.float32)
    d = np.asarray(d, dtype=np.float32)
    W = np.asarray(W, dtype=np.float32)
    sx = np.asarray(sx, dtype=np.int32)
    sy = np.asarray(sy, dtype=np.int32)

    # Host-side gather of the K station feature vectors + the tiny (K,F)@(F,F)
    # matmul (replicated to all cores per the sharding strategy), packed
    # [kp, kc, f] for single-line-per-partition DMA.
    y16 = (x[sx, sy] @ W).astype(np.float16)
    y16 = np.ascontiguousarray(
        y16.reshape(2, 128, F).transpose(1, 0, 2)
    ).reshape(128, 2 * F)

    # Pack d k-major per core: dt[c, s, kp, kc, r] = d[row s*RS + r, kc*128+kp]
    d16 = d.astype(np.float16)
    dv = d16.reshape(NCORES, NSLAB, RS, 2, 128)  # [c, s, r, kc, kp]
    dt = np.ascontiguousarray(dv.transpose(0, 1, 4, 3, 2))

    nc = _build_program()

    in_maps = []
    for c in range(NCORES):
        in_maps.append(
            {
                "d_t": dt[c].reshape(NSLAB * 128, 2 * RS),
                "y_mat": y16,
            }
        )

    res = run_bass_kernel_spmd(nc, in_maps, list(range(NCORES)))
    global last_results
    last_results = res
    out = np.concatenate(
        [r["out_shard"].T.reshape(HS, WG, F) for r in res.results], axis=0
    ).astype(np.float32)
    return out


if __name__ == "__main__":
    rng = np.random.default_rng(0)
    x = rng.standard_normal((H, WG, F), dtype=np.float32)
    d = rng.random((H, WG, K), dtype=np.float32)
    W = rng.standard_normal((K, F), dtype=np.float32) / np.sqrt(F)
    sx = rng.integers(0, H, size=(K,)).astype(np.int32)
    sy = rng.integers(0, WG, size=(K,)).astype(np.int32)
    out = kernel(x, d, W, sx, sy)
    y = x[sx, sy].astype(np.float64) @ W.astype(np.float64)
    exp = d.reshape(-1, K).astype(np.float64) @ y
    exp = exp.reshape(H, WG, F)
    err = np.linalg.norm(out - exp) / np.linalg.norm(exp)
    print("rel err:", err)
